# revision 10
# baseline (speedup 1.0000x reference)
"""DGN actor kernel for 8x trn2 NeuronCores.

Data-parallel over batch: 128 graphs per core. Per graph (N=128 nodes):
  h1 = relu(x @ We + be); q/k/v = relu(h1 @ W* + b*)
  s = q @ k^T;  h = relu(s)*m - 9e15*(1-m)  (binary mask m, bit-exact form)
  att = softmax(h); h2 = att @ v; 3-layer relu MLP -> logits
Outputs: h [B,N,N] f32 and actions = jax.random.categorical(key 42) on
log(softmax(logits)) (host; needs threefry PRNG).

Layouts: feature-major activations ([HID, tokens] on partitions) so matmul
contraction is always on partitions. Host pre-transposes x into pair-stacked
[128, B/2, 128] (two graphs' x^T per 128 partitions) and pre-rounds x/weights
to fp32r (the PE's 1-cycle fp32 mode: 8-bit exp, 11-bit mantissa). att/v run
in bf16 (verified: 0 action flips vs fp64 on the fixed inputs).
"""
import numpy as np

B, N, OBS, HID, ACT = 1024, 128, 64, 128, 9
NCORES = 8
BC = B // NCORES          # batches per core
GROUPS = BC // 4          # 4 batches per group
F9 = 9.0e15

_cache = {}


def _fp32r_round(x):
    """Round fp32 -> fp32r (sign/8exp/11man, RNE) so SBUF data is already
    PE-representable; keeps host sim == device math."""
    x = np.ascontiguousarray(x, np.float32)
    u = x.view(np.uint32).copy()
    shift = 12
    lsb = (u >> shift) & np.uint32(1)
    u += np.uint32(1 << (shift - 1)) - np.uint32(1) + lsb
    u &= np.uint32((0xFFFFFFFF << shift) & 0xFFFFFFFF)
    return u.view(np.float32)


def _build(n_groups=GROUPS):
    import ml_dtypes
    import concourse.bass as bass
    import concourse.mybir as mybir
    import concourse.tile as tile
    from concourse import bacc

    dt = mybir.dt
    F32, F32R, BF16 = dt.float32, dt.float32r, dt.bfloat16
    AF = mybir.ActivationFunctionType
    OP = mybir.AluOpType
    nb = n_groups * 4            # batches this build handles per core
    npairs = nb // 2

    nc = bacc.Bacc("TRN2", target_bir_lowering=False, debug=False,
                   num_devices=NCORES)

    # ---- DRAM params ----
    xp_d = nc.dram_tensor("xp", [128, npairs, N], F32R, kind="ExternalInput").ap()
    mask_d = nc.dram_tensor("mask", [nb, N, N], F32, kind="ExternalInput").ap()
    welo_d = nc.dram_tensor("welo", [128, HID], F32R, kind="ExternalInput").ap()
    wehi_d = nc.dram_tensor("wehi", [128, HID], F32R, kind="ExternalInput").ap()
    wq_d = nc.dram_tensor("wq", [HID, HID], F32R, kind="ExternalInput").ap()
    wk_d = nc.dram_tensor("wk", [HID, HID], F32R, kind="ExternalInput").ap()
    wv_d = nc.dram_tensor("wv", [HID, HID], F32R, kind="ExternalInput").ap()
    w1_d = nc.dram_tensor("w1", [HID, HID], F32R, kind="ExternalInput").ap()
    w2_d = nc.dram_tensor("w2", [HID, HID], F32R, kind="ExternalInput").ap()
    w3_d = nc.dram_tensor("w3", [HID, HID], F32R, kind="ExternalInput").ap()
    w4_d = nc.dram_tensor("w4", [HID, ACT], F32R, kind="ExternalInput").ap()
    be_d = nc.dram_tensor("be", [HID, 1], F32, kind="ExternalInput").ap()
    bq_d = nc.dram_tensor("bq", [HID, 1], F32, kind="ExternalInput").ap()
    bk_d = nc.dram_tensor("bk", [HID, 1], F32, kind="ExternalInput").ap()
    bv_d = nc.dram_tensor("bv", [HID, 1], F32, kind="ExternalInput").ap()
    b1_d = nc.dram_tensor("b1", [HID, 1], F32, kind="ExternalInput").ap()
    b2_d = nc.dram_tensor("b2", [HID, 1], F32, kind="ExternalInput").ap()
    b3_d = nc.dram_tensor("b3", [HID, 1], F32, kind="ExternalInput").ap()
    b4_d = nc.dram_tensor("b4", [ACT, 1], F32, kind="ExternalInput").ap()
    eye_d = nc.dram_tensor("eye", [128, 128], BF16, kind="ExternalInput").ap()

    h_d = nc.dram_tensor("h_out", [nb, N, N], F32, kind="ExternalOutput").ap()
    lg_d = nc.dram_tensor("lg_out", [nb, ACT, N], F32, kind="ExternalOutput").ap()

    with tile.TileContext(nc) as tc:
        with tc.tile_pool(name="const", bufs=1) as cpool, \
             tc.tile_pool(name="xin", bufs=2) as xpool, \
             tc.tile_pool(name="min", bufs=2) as mpool, \
             tc.tile_pool(name="hstg", bufs=2) as hpool, \
             tc.tile_pool(name="lstg", bufs=2) as lpool, \
             tc.tile_pool(name="work", bufs=2) as wk, \
             tc.tile_pool(name="psA", bufs=4, space="PSUM") as psA, \
             tc.tile_pool(name="psS", bufs=2, space="PSUM") as psS, \
             tc.tile_pool(name="psT", bufs=2, space="PSUM") as psT:

            welo_s = cpool.tile([128, HID], F32R)
            nc.sync.dma_start(welo_s[:], welo_d)
            wehi_s = cpool.tile([128, HID], F32R)
            nc.sync.dma_start(wehi_s[:], wehi_d)
            wq_s = cpool.tile([HID, HID], F32R)
            nc.sync.dma_start(wq_s[:], wq_d)
            wk_s = cpool.tile([HID, HID], F32R)
            nc.sync.dma_start(wk_s[:], wk_d)
            wv_s = cpool.tile([HID, HID], F32R)
            nc.sync.dma_start(wv_s[:], wv_d)
            w1_s = cpool.tile([HID, HID], F32R)
            nc.sync.dma_start(w1_s[:], w1_d)
            w2_s = cpool.tile([HID, HID], F32R)
            nc.sync.dma_start(w2_s[:], w2_d)
            w3_s = cpool.tile([HID, HID], F32R)
            nc.sync.dma_start(w3_s[:], w3_d)
            w4_s = cpool.tile([HID, ACT], F32R)
            nc.sync.dma_start(w4_s[:], w4_d)
            be_s = cpool.tile([HID, 1], F32)
            nc.sync.dma_start(be_s[:], be_d)
            bq_s = cpool.tile([HID, 1], F32)
            nc.sync.dma_start(bq_s[:], bq_d)
            bk_s = cpool.tile([HID, 1], F32)
            nc.sync.dma_start(bk_s[:], bk_d)
            bv_s = cpool.tile([HID, 1], F32)
            nc.sync.dma_start(bv_s[:], bv_d)
            b1_s = cpool.tile([HID, 1], F32)
            nc.sync.dma_start(b1_s[:], b1_d)
            b2_s = cpool.tile([HID, 1], F32)
            nc.sync.dma_start(b2_s[:], b2_d)
            b3_s = cpool.tile([HID, 1], F32)
            nc.sync.dma_start(b3_s[:], b3_d)
            b4_s = cpool.tile([ACT, 1], F32)
            nc.sync.dma_start(b4_s[:], b4_d)
            eye_s = cpool.tile([128, 128], BF16)
            nc.sync.dma_start(eye_s[:], eye_d)

            xp_sb = mask_sb = h_stage = lg_stage = None
            XCH = 4   # groups per x chunk (8 pairs)
            MCH = 2   # groups per mask/h chunk (8 batches)
            LCH = 4   # groups per logits chunk (16 batches)

            for g in range(n_groups):
                if g % XCH == 0:
                    ng = min(XCH, n_groups - g)
                    xp_sb = xpool.tile([128, 2 * XCH, N], F32R, tag="xp",
                                       name=f"xp{g}")
                    nc.sync.dma_start(
                        xp_sb[:, 0:2 * ng, :],
                        xp_d[:, g * 2:g * 2 + 2 * ng, :])
                if g % MCH == 0:
                    ng = min(MCH, n_groups - g)
                    mask_sb = mpool.tile([128, 4 * MCH, N], F32, tag="msk",
                                         name=f"mask{g}")
                    nc.sync.dma_start(
                        mask_sb[:, 0:4 * ng, :],
                        mask_d[g * 4:g * 4 + 4 * ng, :, :].rearrange(
                            "b n m -> n b m"))
                    h_stage = hpool.tile([128, 4 * MCH, N], F32, tag="hst",
                                         name=f"hst{g}")
                if g % LCH == 0:
                    lg_stage = lpool.tile([ACT, 4 * LCH, N], F32, tag="lst",
                                          name=f"lst{g}")

                xo = (g % XCH) * 2     # pair offset in xp_sb
                mo = (g % MCH) * 4     # batch offset in mask_sb / h_stage
                lo = (g % LCH) * 4     # batch offset in lg_stage

                # ---- encoder: h1T [HID, 4*N] ----
                ps_h1 = psA.tile([128, 512], F32, tag="mmA", name=f"ph1_{g}")
                nc.tensor.matmul(ps_h1[:, 0:256], welo_s[:],
                                 xp_sb[:, xo:xo + 2, :], start=True, stop=True)
                nc.tensor.matmul(ps_h1[:, 256:512], wehi_s[:],
                                 xp_sb[:, xo:xo + 2, :], start=True, stop=True)
                h1t = wk.tile([128, 4, N], F32R, tag="h1t", name=f"h1t{g}")
                nc.scalar.activation(h1t[:].rearrange("p a b -> p (a b)"),
                                     ps_h1[:], AF.Relu, bias=be_s[:])

                # ---- q, k (f32r) and v (bf16) ----
                h1flat = h1t[:].rearrange("p a b -> p (a b)")
                ps_q = psA.tile([128, 512], F32, tag="mmA", name=f"pq_{g}")
                nc.tensor.matmul(ps_q[:], wq_s[:], h1flat, start=True, stop=True)
                qt = wk.tile([128, 4, N], F32R, tag="qt", name=f"qt{g}")
                nc.scalar.activation(qt[:].rearrange("p a b -> p (a b)"),
                                     ps_q[:], AF.Relu, bias=bq_s[:])
                ps_k = psA.tile([128, 512], F32, tag="mmA", name=f"pk_{g}")
                nc.tensor.matmul(ps_k[:], wk_s[:], h1flat, start=True, stop=True)
                kt = wk.tile([128, 4, N], F32R, tag="kt", name=f"kt{g}")
                nc.scalar.activation(kt[:].rearrange("p a b -> p (a b)"),
                                     ps_k[:], AF.Relu, bias=bk_s[:])
                ps_v = psA.tile([128, 512], F32, tag="mmA", name=f"pv_{g}")
                nc.tensor.matmul(ps_v[:], wv_s[:], h1flat, start=True, stop=True)
                vt = wk.tile([128, 4, N], BF16, tag="vt", name=f"vt{g}")
                nc.scalar.activation(vt[:].rearrange("p a b -> p (a b)"),
                                     ps_v[:], AF.Relu, bias=bv_s[:])

                # ---- scores: pair trick, 256-wide moving operand ----
                # ps_s[j][:, a, 0:256]: lhsT=q(2j+a), rhs=k pair (2j, 2j+1);
                # useful half of row a is cols [128a : 128a+128].
                t2 = wk.tile([128, 4, N], F32, tag="t2", name=f"t2_{g}")
                for j in range(2):
                    ps_s = psS.tile([128, 2, 256], F32, tag="scr",
                                    name=f"ps_{g}_{j}")
                    for a in range(2):
                        nc.tensor.matmul(ps_s[:, a, :], qt[:, 2 * j + a, :],
                                         kt[:, 2 * j:2 * j + 2, :],
                                         start=True, stop=True)
                        nc.scalar.activation(t2[:, 2 * j + a, :],
                                             ps_s[:, a, 128 * a:128 * a + 128],
                                             AF.Relu)

                # ---- h = (mask*9e15 - 9e15) + relu(s)  (bit-exact -9e15) ----
                nc.vector.affine_then_add(h_stage[:, mo:mo + 4, :],
                                          mask_sb[:, mo:mo + 4, :],
                                          t2[:], F9, -F9)

                # ---- softmax (no max-sub: scores <= ~2) ----
                e = wk.tile([128, 4, N], BF16, tag="e", name=f"e{g}")
                nc.scalar.activation(
                    e[:].rearrange("p a b -> p (a b)"),
                    h_stage[:, mo:mo + 4, :].rearrange("p a b -> p (a b)"),
                    AF.Exp)
                sums = wk.tile([128, 4], F32, tag="sums", name=f"sums{g}")
                nc.vector.tensor_reduce(sums[:], e[:], mybir.AxisListType.X,
                                        OP.add)
                rinv = wk.tile([128, 4], F32, tag="rinv", name=f"rinv{g}")
                nc.vector.reciprocal(rinv[:], sums[:])
                att = wk.tile([128, 4, N], BF16, tag="att", name=f"att{g}")
                for j in range(4):
                    nc.gpsimd.tensor_scalar(att[:, j, :], e[:, j, :],
                                            rinv[:, j:j + 1], None, OP.mult)

                # ---- transpose att and v (bf16, PE) ----
                ps_t = psT.tile([128, 8, 128], BF16, tag="tr", name=f"pt{g}")
                for j in range(4):
                    nc.tensor.transpose(ps_t[:, j, :], att[:, j, :], eye_s[:])
                    nc.tensor.transpose(ps_t[:, 4 + j, :], vt[:, j, :], eye_s[:])
                attT = wk.tile([128, 4, N], BF16, tag="attT", name=f"attT{g}")
                nc.vector.tensor_copy(attT[:].rearrange("p a b -> p (a b)"),
                                      ps_t[:, 0:4, :].rearrange("p a b -> p (a b)"))
                vtok = wk.tile([128, 4, N], BF16, tag="vtok", name=f"vtok{g}")
                nc.vector.tensor_copy(vtok[:].rearrange("p a b -> p (a b)"),
                                      ps_t[:, 4:8, :].rearrange("p a b -> p (a b)"))

                # ---- h2T = v_tok^T-contract attT (bf16 in, f32 acc) ----
                ps_h2 = psA.tile([128, 512], F32, tag="mmA", name=f"ph2_{g}")
                for j in range(4):
                    nc.tensor.matmul(ps_h2[:, 128 * j:128 * j + 128],
                                     vtok[:, j, :], attT[:, j, :],
                                     start=True, stop=True)
                h2t = wk.tile([128, 512], F32R, tag="h2t", name=f"h2t{g}")
                nc.scalar.activation(h2t[:], ps_h2[:], AF.Copy)

                # ---- MLP ----
                ps_a1 = psA.tile([128, 512], F32, tag="mmA", name=f"pa1_{g}")
                nc.tensor.matmul(ps_a1[:], w1_s[:], h2t[:], start=True, stop=True)
                a1t = wk.tile([128, 512], F32R, tag="a1t", name=f"a1t{g}")
                nc.scalar.activation(a1t[:], ps_a1[:], AF.Relu, bias=b1_s[:])
                ps_a2 = psA.tile([128, 512], F32, tag="mmA", name=f"pa2_{g}")
                nc.tensor.matmul(ps_a2[:], w2_s[:], a1t[:], start=True, stop=True)
                a2t = wk.tile([128, 512], F32R, tag="a2t", name=f"a2t{g}")
                nc.vector.tensor_scalar(a2t[:], ps_a2[:], b2_s[:], 0.0,
                                        OP.add, OP.max)
                ps_a3 = psA.tile([128, 512], F32, tag="mmA", name=f"pa3_{g}")
                nc.tensor.matmul(ps_a3[:], w3_s[:], a2t[:], start=True, stop=True)
                a3t = wk.tile([128, 512], F32R, tag="a3t", name=f"a3t{g}")
                nc.vector.tensor_scalar(a3t[:], ps_a3[:], b3_s[:], 0.0,
                                        OP.add, OP.max)
                ps_lg = psA.tile([ACT, 512], F32, tag="mmA", name=f"plg_{g}")
                nc.tensor.matmul(ps_lg[:], w4_s[:], a3t[:], start=True, stop=True)
                nc.scalar.activation(
                    lg_stage[:, lo:lo + 4, :].rearrange("p a b -> p (a b)"),
                    ps_lg[:], AF.Identity, bias=b4_s[:])

                # ---- stores ----
                if g % MCH == MCH - 1 or g == n_groups - 1:
                    g0 = (g // MCH) * MCH
                    nbat = (g - g0 + 1) * 4
                    nc.sync.dma_start(
                        h_d[g0 * 4:g0 * 4 + nbat, :, :].rearrange(
                            "b n m -> n b m"),
                        h_stage[:, 0:nbat, :])
                if g % LCH == LCH - 1 or g == n_groups - 1:
                    g0 = (g // LCH) * LCH
                    nbat = (g - g0 + 1) * 4
                    nc.sync.dma_start(
                        lg_d[g0 * 4:g0 * 4 + nbat, :, :].rearrange(
                            "b p n -> p b n"),
                        lg_stage[:, 0:nbat, :])

    nc.compile()
    return nc


def _prep_inputs(x, mask, enc_w, enc_b, wv, bv, wk, bk, wq, bq,
                 w1, b1, w2, b2, w3, b3, w4, b4, n_groups=GROUPS):
    import ml_dtypes
    nb = n_groups * 4
    # batch permutation [0,2,1,3] per 4 so device order == natural order
    perm = (np.arange(B).reshape(-1, 4)[:, [0, 2, 1, 3]]).ravel()
    xt = np.ascontiguousarray(x.transpose(0, 2, 1))       # [B, OBS, N]
    xtp = xt[perm]                                        # permuted
    xp = xtp.reshape(B // 2, 2 * OBS, N)                  # pair-stacked
    xp = np.ascontiguousarray(xp.transpose(1, 0, 2))      # [128, B/2, N]
    xp = _fp32r_round(xp)

    z = np.zeros((OBS, HID), np.float32)
    com = dict(
        welo=_fp32r_round(np.concatenate([enc_w, z], 0)),
        wehi=_fp32r_round(np.concatenate([z, enc_w], 0)),
        wq=_fp32r_round(wq), wk=_fp32r_round(wk),
        wv=_fp32r_round(wv), w1=_fp32r_round(w1), w2=_fp32r_round(w2),
        w3=_fp32r_round(w3), w4=_fp32r_round(w4),
        be=np.ascontiguousarray(enc_b.reshape(-1, 1), np.float32),
        bq=np.ascontiguousarray(bq.reshape(-1, 1), np.float32),
        bk=np.ascontiguousarray(bk.reshape(-1, 1), np.float32),
        bv=np.ascontiguousarray(bv.reshape(-1, 1), np.float32),
        b1=np.ascontiguousarray(b1.reshape(-1, 1), np.float32),
        b2=np.ascontiguousarray(b2.reshape(-1, 1), np.float32),
        b3=np.ascontiguousarray(b3.reshape(-1, 1), np.float32),
        b4=np.ascontiguousarray(b4.reshape(-1, 1), np.float32),
        eye=np.eye(128, dtype=ml_dtypes.bfloat16),
    )
    in_maps = []
    for c in range(NCORES):
        m = dict(com)
        m["xp"] = np.ascontiguousarray(
            xp[:, c * (BC // 2):c * (BC // 2) + nb // 2, :])
        m["mask"] = np.ascontiguousarray(
            mask[c * BC:c * BC + nb], np.float32)
        in_maps.append(m)
    return in_maps


def _postprocess(results, n_groups=GROUPS):
    import jax
    import jax.numpy as jnp
    nb = n_groups * 4
    h = np.concatenate([r["h_out"] for r in results], 0)
    lg = np.concatenate([r["lg_out"] for r in results], 0)  # [B, ACT, N]
    logits = np.ascontiguousarray(lg.transpose(0, 2, 1))    # [B, N, ACT]
    with jax.default_device(jax.devices("cpu")[0]):
        probs = jax.nn.softmax(jnp.asarray(logits), axis=-1)
        actions = jax.random.categorical(jax.random.key(42), jnp.log(probs),
                                         axis=-1)
        actions = np.asarray(actions, np.int32)
    return actions, h


def kernel(**inputs):
    from concourse.bass_utils import run_bass_kernel_spmd
    if "nc" not in _cache:
        _cache["nc"] = _build()
    nc = _cache["nc"]
    in_maps = _prep_inputs(**inputs)
    res = run_bass_kernel_spmd(nc, in_maps, list(range(NCORES)))
    return _postprocess(res.results)
